# revision 1
# baseline (speedup 1.0000x reference)
"""Trainium2 Bass kernel for nn_CCN1D (circulant GNN message passing).

Strategy
--------
The reference gathers receptive fields on a circulant ring graph and runs
per-edge MLPs followed by segment sums.  Because every gathered row's MLP
output depends only on the *source* vertex, the per-edge MLPs (130k / 250k
rows) collapse to per-vertex MLPs (10k rows) plus sliding-window sums along
the ring:

    dense = relu(X @ W1 + b1)                           [N, 128]
    z_f[u]  = relu(relu(dense[u] @ (w0a_lo+w0a_hi)/13) @ w0b)      [N, 64]
    s0_f[v] = sum_{j=0..12} z_f[(v+j) % N]              (window sum)
    z1_f[u] = relu(relu(concat(s0_f[u], z_f[u])/25 @ w1a) @ w1b)
    s1_f[v] = sum_{j=0..24} z1_f[(v+j) % N]
    (reverse branch identical with backward windows)
    logits  = concat(dense, s0f, s1f, s0r, s1r) @ W2 + b2
    out     = log_softmax(logits) * mask

Sharding: vertices are range-partitioned across 8 cores with a 36-vertex
halo on each side (graph/data parallel; weights replicated; no device
collectives needed - the halo makes every core self-sufficient).

On-chip layout is feature-major ([feature partitions, vertex-lane free dim])
so every matmul contracts over partitions, and the window sums become
prefix-scan + shifted-subtract along the free dimension.

v2 redesign (vs the f32r baseline):
- whole datapath in bf16 (PSUM stays f32, scans stay f32): halves HBM
  traffic, bf16 matmuls run 1 col/cycle, and rel err stays ~2e-3.
- the reverse branch's z is stored pre-shifted (+12 lanes for layer 0,
  +24 for layer 1) so ONE [128,W] prefix scan and ONE [128,W] shifted
  subtract produce both branches' window sums.
- fc2 packs [s0f;s0r] and [s1f;s1r] into 128-contract tiles: 3 matmuls
  instead of 5, over the 1250 owned lanes only.
- 10 transposes at lane offset 36 (owned region starts at a tile edge),
  trivial output DMA.
- layer-1's 128-contract input [s0; z] is never materialized: two
  64-contract matmuls accumulate from S0 and Z directly (the reverse
  branch reads z through its +12 shifted view).
- element-wise work balanced: Act = f-branch + fc1/fc2 evictions;
  DVE = r-branch evictions, scans, window subtracts (chained on one
  queue), softmax; fc2 col-chunks aligned to the subtract staircase so
  each chunk waits only on its own window subtract.
- optional tc.For_i hardware loop (timing builds) repeats the body
  on-device without instruction-count blowup.
"""

import sys

import numpy as np

for _p in ("/opt/trn_rl_repo",):
    if _p not in sys.path:
        sys.path.insert(0, _p)

N = 10000
NCORES = 8
BLK = N // NCORES          # 1250 vertices per core
HALO = 36                  # 12 (layer-0 window) + 24 (layer-1 window)
W = 1344                   # on-chip free width (1322 valid + pad)
NT = 10                    # 128-lane transpose tiles at lane offset 36
CTS = ((0, 512), (512, 512), (1024, 320))        # layer-0 col tiles
CTS1 = ((0, 500), (500, 512), (1012, 320))       # layer-1 col tiles
SUB13 = ((1, 500), (500, 1012), (1012, 1332))    # S0 = P13[+12] - P13[-1]
SUB25 = ((1, 476), (476, 988), (988, 1320))      # S1 = P25[+24] - P25[-1]
CTS2 = ((36, 476), (476, 988), (988, 1316))     # fc2 over owned lanes,
                                                 # aligned to SUB25 so chunk
                                                 # j waits only on sub25 j
                                                 # (to 1316 so all 10
                                                 # transposes are 128 wide)
RF1, RF2 = 13, 25
C_IN, C_HID, MLP_H, MSG, NCLS = 512, 128, 128, 64, 16
LO, HI = HALO, HALO + BLK  # valid output lane range [36, 1286)
WPACK_COLS = 1356          # packed bf16 weights + biases + mask, one DMA
WARM_GROUPS = 6            # dummy PE groups to ramp the clock during DMA

_F32 = np.float32


# --------------------------------------------------------------------------
# structure check (is the input the circulant graph the kernel was built for?)
# --------------------------------------------------------------------------

def _expected_idx():
    v = np.arange(N)
    return {
        "f_rf1": ((v[:, None] + np.arange(RF1)) % N).reshape(-1),
        "f_rf2": ((v[:, None] + np.arange(RF2)) % N).reshape(-1),
        "r_rf1": ((v[:, None] - np.arange(RF1)) % N).reshape(-1),
        "r_rf2": ((v[:, None] - np.arange(RF2)) % N).reshape(-1),
        "own1": np.repeat(v, RF1),
        "own2": np.repeat(v, RF2),
        "self1": v * RF1,
    }


def _structure_matches(inputs):
    try:
        if inputs["sparse_feature"].shape != (N, C_IN):
            return False
        for k, exp in _expected_idx().items():
            got = np.asarray(inputs[k])
            if got.shape != exp.shape or not np.array_equal(got, exp):
                return False
        return True
    except Exception:
        return False


# --------------------------------------------------------------------------
# generic numpy fallback (exact reference semantics, any index content)
# --------------------------------------------------------------------------

def _segment_sum(data, seg, num):
    out = np.zeros((num,) + data.shape[1:], dtype=data.dtype)
    np.add.at(out, seg, data)
    return out


def _np_branch(dense, rf1, rf2, own1, own2, self1, w0a, w0b, w1a, w1b):
    sizes1 = _segment_sum(np.ones(own1.shape, dense.dtype), own1, N)
    sizes2 = _segment_sum(np.ones(own2.shape, dense.dtype), own2, N)
    g = dense[rf1]
    m0 = np.concatenate([g, g], axis=-1) / sizes1[own1][:, None]
    h0 = np.maximum(np.maximum(m0 @ w0a, 0.0) @ w0b, 0.0)
    s0 = _segment_sum(h0, own1, N)
    selfr = h0[self1]
    m1 = np.concatenate([s0[rf2], selfr[rf2]], axis=-1) / sizes2[own2][:, None]
    h1 = np.maximum(np.maximum(m1 @ w1a, 0.0) @ w1b, 0.0)
    s1 = _segment_sum(h1, own2, N)
    return s0, s1


def _reference_numpy(inputs):
    f = {k: np.asarray(v) for k, v in inputs.items()}
    dense = np.maximum(
        f["sparse_feature"].astype(_F32) @ f["fc1_w"] + f["fc1_b"], 0.0
    )
    s0f, s1f = _np_branch(dense, f["f_rf1"], f["f_rf2"], f["own1"], f["own2"],
                          f["self1"], f["mw0a"], f["mw0b"], f["mw1a"], f["mw1b"])
    s0r, s1r = _np_branch(dense, f["r_rf1"], f["r_rf2"], f["own1"], f["own2"],
                          f["self1"], f["rw0a"], f["rw0b"], f["rw1a"], f["rw1b"])
    total = np.concatenate([dense, s0f, s1f, s0r, s1r], axis=1)
    logits = total @ f["fc2_w"] + f["fc2_b"]
    m = logits.max(axis=-1, keepdims=True)
    lse = m + np.log(np.exp(logits - m).sum(axis=-1, keepdims=True))
    return ((logits - lse) * f["mask"][:, None].astype(_F32)).astype(_F32)


# --------------------------------------------------------------------------
# device kernel
# --------------------------------------------------------------------------

_NC = None


def _build_nc(repeat=1, hw_loop=0):
    import concourse.bass as bass
    import concourse.tile as tile
    from concourse import bacc, mybir
    from contextlib import ExitStack, nullcontext

    f32 = mybir.dt.float32
    bf16 = mybir.dt.bfloat16
    AF = mybir.ActivationFunctionType
    OP = mybir.AluOpType

    nc = bacc.Bacc(trn_type="TRN2", debug=False)

    xt_d = nc.dram_tensor("xt", [C_IN, W], bf16, kind="ExternalInput").ap()
    wpack_d = nc.dram_tensor("wpack", [128, WPACK_COLS], bf16,
                             kind="ExternalInput").ap()
    out_d = nc.dram_tensor("out", [BLK, NCLS], f32, kind="ExternalOutput").ap()

    with tile.TileContext(nc) as tc:
        with ExitStack() as ctx:
            cp = ctx.enter_context(tc.tile_pool(name="consts", bufs=1))
            ap_ = ctx.enter_context(tc.tile_pool(name="acts", bufs=1))
            sp = ctx.enter_context(tc.tile_pool(name="scr", bufs=8))
            pmm = ctx.enter_context(tc.tile_pool(name="pmm", bufs=3, space="PSUM"))
            pz = ctx.enter_context(tc.tile_pool(name="pz", bufs=2, space="PSUM"))
            pl = ctx.enter_context(tc.tile_pool(name="pl", bufs=2, space="PSUM"))
            pt = ctx.enter_context(tc.tile_pool(name="pt", bufs=1, space="PSUM"))

            def emit():
                # ---- input DMAs: consts, weights, then X per (tile, k) ----
                wpack = cp.tile([128, WPACK_COLS], bf16, tag="wpack", name="wpack")
                nc.sync.dma_start(out=wpack, in_=wpack_d)

                xt_pack = cp.tile([128, 4, W], bf16, tag="xtp", name="xt_pack")
                xt = [xt_pack[:, k, :] for k in range(4)]
                xt_k = xt_d.rearrange("(k p) w -> p k w", p=128)
                for s, w in CTS:
                    for k in range(4):
                        nc.sync.dma_start(out=xt_pack[:, k, s:s + w],
                                          in_=xt_k[:, k, s:s + w])

                # PE warm-up on a zeroed scratch tile (no DMA dependency):
                # keeps the HAM clock gate ramping while X streams in.
                if WARM_GROUPS:
                    wrm = cp.tile([128, 512], bf16, tag="wrm", name="wrm")
                    nc.vector.memset(wrm, 0.0)
                    warm = pl.tile([NCLS, 512], f32, tag="psL", name="warm")
                    for i in range(WARM_GROUPS):
                        nc.tensor.matmul(warm, wrm[:, 0:NCLS], wrm,
                                         start=(i == 0),
                                         stop=(i == WARM_GROUPS - 1),
                                         skip_group_check=True)

                wfc1 = [wpack[:, 128 * k:128 * (k + 1)] for k in range(4)]
                wz = {"f": wpack[:, 512:640], "r": wpack[:, 640:768]}
                wzb = {"f": wpack[:, 768:832], "r": wpack[:, 832:896]}
                # layer-1 weight halves: cols 896:1024 hold the s0 (lo) half,
                # cols 1024:1152 the z (hi) half; partitions 0:64 = forward
                # branch, 64:128 = reverse branch (matching S0/Z layout).
                wz1lo = {"f": wpack[0:64, 896:1024],
                         "r": wpack[64:128, 896:1024]}
                wz1hi = {"f": wpack[0:64, 1024:1152],
                         "r": wpack[64:128, 1024:1152]}
                wz1b = {"f": wpack[:, 1152:1216], "r": wpack[:, 1216:1280]}
                w2 = [wpack[:, 1280:1296], wpack[:, 1296:1312],
                      wpack[:, 1312:1328]]
                ident = wpack[0:16, 1328:1344]
                bfc1 = wpack[:, 1344:1345]
                bfc2 = wpack[0:16, 1345:1346]
                maskv = wpack[:, 1346:1346 + NT]

                # ---- persistent activation tiles (bf16; scans f32) ----
                D = ap_.tile([128, W], bf16, tag="D")
                Z = ap_.tile([128, W], bf16, tag="Z")    # [0:64]=z_f, [64:]=z_r>>12
                Z1 = ap_.tile([128, W], bf16, tag="Z1")  # [0:64]=z1_f, [64:]=z1_r>>24
                S0 = ap_.tile([128, W], bf16, tag="S0")  # [0:64]=s0f, [64:]=s0r
                S1 = ap_.tile([128, W], bf16, tag="S1")  # [0:64]=s1f, [64:]=s1r
                P13 = ap_.tile([128, W], f32, tag="P13")  # prefix sums of Z
                P25 = ap_.tile([128, W], f32, tag="P25")  # prefix sums of Z1
                Lsb = ap_.tile([NCLS, W], bf16, tag="Lsb")
                LT = ap_.tile([128, NT, NCLS], f32, tag="LT")

                # edge-lane zero patches (regions no eviction reaches)
                nc.gpsimd.memset(Z[64:128, 0:12], 0.0)
                nc.gpsimd.memset(Z1[64:128, 0:24], 0.0)
                nc.gpsimd.memset(P25[:, 1332:W], 0.0)

                # ---- stage A+B per col-tile: fc1, layer-0 MLPs, chained scan
                for j, (s, w) in enumerate(CTS):
                    psA = pmm.tile([128, 512], f32, tag="mm", name="psA")
                    for k in range(4):
                        nc.tensor.matmul(psA[:, :w], wfc1[k], xt[k][:, s:s + w],
                                         start=(k == 0), stop=(k == 3))
                    nc.scalar.activation(D[:, s:s + w], psA[:, :w], AF.Relu,
                                         bias=bfc1)
                    # forward branch: Act evictions
                    t1 = pmm.tile([128, 512], f32, tag="mm", name="t1")
                    nc.tensor.matmul(t1[:, :w], wz["f"], D[:, s:s + w],
                                     start=True, stop=True)
                    t1s = sp.tile([128, 512], bf16, tag="t1s", name="t1sf")
                    nc.scalar.activation(t1s[:, :w], t1[:, :w], AF.Relu)
                    zp = pz.tile([64, 512], f32, tag="zz", name="zpf")
                    nc.tensor.matmul(zp[:, :w], wzb["f"], t1s[:, :w],
                                     start=True, stop=True)
                    nc.scalar.activation(Z[0:64, s:s + w], zp[:, :w], AF.Relu)
                    # reverse branch: DVE evictions; z_r stored shifted +12
                    t1r = pmm.tile([128, 512], f32, tag="mm", name="t1r")
                    nc.tensor.matmul(t1r[:, :w], wz["r"], D[:, s:s + w],
                                     start=True, stop=True)
                    t1rs = sp.tile([128, 512], bf16, tag="t1s", name="t1sr")
                    nc.vector.tensor_scalar_max(t1rs[:, :w], t1r[:, :w], 0.0)
                    zpr = pz.tile([64, 512], f32, tag="zz", name="zpr")
                    nc.tensor.matmul(zpr[:, :w], wzb["r"], t1rs[:, :w],
                                     start=True, stop=True)
                    wc = min(s + w + 12, W) - (s + 12)
                    nc.vector.tensor_scalar_max(Z[64:128, s + 12:s + 12 + wc],
                                                zpr[:, :wc], 0.0)
                    # chained prefix scan over both branches at once, then
                    # this tile's staircase window-subtract (its P13 reads
                    # stay within scans <= j, so it can issue immediately)
                    nc.vector.tensor_tensor_scan(
                        P13[:, s:s + w], Z[:, s:s + w], Z[:, s:s + w],
                        initial=(0.0 if s == 0 else P13[:, s - 1:s]),
                        op0=OP.add, op1=OP.bypass)
                    lo, hi = SUB13[j]
                    nc.vector.tensor_sub(S0[:, lo:hi], P13[:, lo + 12:hi + 12],
                                         P13[:, lo - 1:hi - 1])
                    if j == 0:
                        nc.vector.tensor_copy(S0[:, 0:1], P13[:, 12:13])

                # ---- stage D: layer-1 MLPs + chained scan.  The 128-contract
                # input [s0; z] is never materialized: two 64-contract matmuls
                # accumulate from S0 and Z directly (z_r via its +12 view).
                for j, (a, w1) in enumerate(CTS1):
                    t2 = pmm.tile([128, 512], f32, tag="mm", name="t2")
                    nc.tensor.matmul(t2[:, :w1], wz1lo["f"], S0[0:64, a:a + w1],
                                     start=True, stop=False)
                    nc.tensor.matmul(t2[:, :w1], wz1hi["f"], Z[0:64, a:a + w1],
                                     start=False, stop=True)
                    t2s = sp.tile([128, 512], bf16, tag="t1s", name="t2sf")
                    nc.scalar.activation(t2s[:, :w1], t2[:, :w1], AF.Relu)
                    z1p = pz.tile([64, 512], f32, tag="zz", name="z1pf")
                    nc.tensor.matmul(z1p[:, :w1], wz1b["f"], t2s[:, :w1],
                                     start=True, stop=True)
                    nc.scalar.activation(Z1[0:64, a:a + w1], z1p[:, :w1],
                                         AF.Relu)
                    t2r = pmm.tile([128, 512], f32, tag="mm", name="t2r")
                    nc.tensor.matmul(t2r[:, :w1], wz1lo["r"], S0[64:128, a:a + w1],
                                     start=True, stop=False)
                    nc.tensor.matmul(t2r[:, :w1], wz1hi["r"],
                                     Z[64:128, a + 12:a + 12 + w1],
                                     start=False, stop=True)
                    t2rs = sp.tile([128, 512], bf16, tag="t1s", name="t2sr")
                    nc.vector.tensor_scalar_max(t2rs[:, :w1], t2r[:, :w1], 0.0)
                    z1pr = pz.tile([64, 512], f32, tag="zz", name="z1pr")
                    nc.tensor.matmul(z1pr[:, :w1], wz1b["r"], t2rs[:, :w1],
                                     start=True, stop=True)
                    wc = min(a + w1 + 24, W) - (a + 24)
                    nc.vector.tensor_scalar_max(Z1[64:128, a + 24:a + 24 + wc],
                                                z1pr[:, :wc], 0.0)
                    nc.vector.tensor_tensor_scan(
                        P25[:, a:a + w1], Z1[:, a:a + w1], Z1[:, a:a + w1],
                        initial=(0.0 if a == 0 else P25[:, a - 1:a]),
                        op0=OP.add, op1=OP.bypass)
                    lo, hi = SUB25[j]
                    nc.vector.tensor_sub(S1[:, lo:hi], P25[:, lo + 24:hi + 24],
                                         P25[:, lo - 1:hi - 1])
                    if j == 0:
                        nc.vector.tensor_copy(S1[:, 0:1], P25[:, 24:25])

                # ---- stages F+G interleaved: each fc2 chunk immediately
                # feeds the transposes it unblocks, and softmax chunk A runs
                # under fc2's last chunk.
                psT = pt.tile([128, NT, NCLS], bf16, tag="psT", name="psT")

                def fc2_chunk(lo, hi):
                    w2w = hi - lo
                    psl = pl.tile([NCLS, 512], f32, tag="psL", name="psl")
                    chunks = ((w2[0], D[:, lo:hi]), (w2[1], S0[:, lo:hi]),
                              (w2[2], S1[:, lo:hi]))
                    for i, (wc2, rhs) in enumerate(chunks):
                        nc.tensor.matmul(psl[:, :w2w], wc2, rhs,
                                         start=(i == 0), stop=(i == 2))
                    nc.scalar.activation(Lsb[:, lo:hi], psl[:, :w2w],
                                         AF.Identity, bias=bfc2)

                def transp(ts):
                    for t in ts:
                        off = LO + 128 * t
                        nc.tensor.transpose(psT[:, t, :],
                                            Lsb[:, off:off + 128], ident)

                def bcast(t2d, n):
                    return bass.AP(tensor=t2d.tensor, offset=t2d.offset,
                                   ap=[t2d.ap[0], [t2d.ap[1][0], n], [0, NCLS]])

                se = sp.tile([128, NT], f32, tag="se", name="se")
                ex = sp.tile([128, NT, NCLS], f32, tag="ex", name="ex")

                def softmax_chunk(t0, nt):
                    # logits are bounded (|L| ~ 2), so exp needs no
                    # max-subtraction: lse = ln(sum(exp(L))) directly.
                    lt = LT[:, t0:t0 + nt, :]
                    ps3 = psT[:, t0:t0 + nt, :]
                    seh = se[:, t0:t0 + nt]
                    nc.scalar.activation(ex[:, t0:t0 + nt, :], ps3, AF.Exp)
                    nc.vector.reduce_sum(seh, ex[:, t0:t0 + nt, :],
                                         axis=mybir.AxisListType.X)
                    nc.scalar.activation(seh, seh, AF.Ln)
                    nc.vector.tensor_sub(lt, ps3, bcast(seh, nt))
                    nc.vector.tensor_mul(lt, lt, bcast(maskv[:, t0:t0 + nt], nt))

                for lo, hi in CTS2:
                    fc2_chunk(lo, hi)
                transp(range(NT))
                softmax_chunk(0, 5)
                midA = out_d[0:640, :].rearrange("(t p) c -> p t c", p=128)
                nc.sync.dma_start(out=midA, in_=LT[:, 0:5, :])
                softmax_chunk(5, 5)
                midB = out_d[640:1152, :].rearrange("(t p) c -> p t c", p=128)
                nc.sync.dma_start(out=midB, in_=LT[:, 5:9, :])
                # tail rows on the idle Pool SWDGE ring, parallel with midB
                nc.gpsimd.dma_start(out=out_d[1152:BLK, :],
                                    in_=LT[0:98, NT - 1, :])

            if hw_loop:
                with tc.For_i(0, hw_loop):
                    emit()
            else:
                for _rep in range(repeat):
                    emit()

    # Steer the ACT-table pass to natural_log_exp_and_others (covers Relu,
    # Identity, Copy, Exp AND Ln) so the kernel pays one table load instead
    # of a ~2.7us mid-kernel switch before the final Ln.
    import concourse.bacc as bacc_mod
    from concourse import mybir as _mb

    AF = _mb.ActivationFunctionType
    orig_tables = bacc_mod.get_activation_tables
    mine = {AF.Relu, AF.Identity, AF.Copy, AF.Exp, AF.Ln}

    def steered(arch):
        t = orig_tables(arch)
        out = {}
        seen_pref = False
        for name, fns in t.items():
            if name == "natural_log_exp_and_others":
                seen_pref = True
                out[name] = fns
            elif not seen_pref:
                out[name] = type(fns)(f for f in fns if f not in mine)
            else:
                out[name] = fns
        return out

    bacc_mod.get_activation_tables = steered
    try:
        nc.compile()
    finally:
        bacc_mod.get_activation_tables = orig_tables
    return nc


def _get_nc(repeat=1, hw_loop=0):
    global _NC
    if repeat != 1 or hw_loop:
        return _build_nc(repeat, hw_loop)
    if _NC is None:
        _NC = _build_nc()
    return _NC


# --------------------------------------------------------------------------
# host-side sharding + entry point
# --------------------------------------------------------------------------

def _make_in_maps(inputs):
    from concourse import mybir

    bf16np = mybir.dt.np(mybir.dt.bfloat16)
    sf = np.ascontiguousarray(np.asarray(inputs["sparse_feature"], dtype=_F32))
    maskf = np.asarray(inputs["mask"]).astype(_F32)

    def f(k):
        return np.asarray(inputs[k], dtype=_F32)

    mw0a, rw0a = f("mw0a"), f("rw0a")
    wpack = np.zeros((128, WPACK_COLS), dtype=_F32)
    wpack[:, 0:512] = f("fc1_w").reshape(4, 128, C_HID).transpose(1, 0, 2) \
        .reshape(128, 512)
    wpack[:, 512:640] = (mw0a[:C_HID] + mw0a[C_HID:]) / RF1
    wpack[:, 640:768] = (rw0a[:C_HID] + rw0a[C_HID:]) / RF1
    wpack[:, 768:832] = f("mw0b")
    wpack[:, 832:896] = f("rw0b")
    mw1a, rw1a = f("mw1a") / RF2, f("rw1a") / RF2
    wpack[0:64, 896:1024] = mw1a[0:64]      # s0 half, forward
    wpack[64:128, 896:1024] = rw1a[0:64]    # s0 half, reverse
    wpack[0:64, 1024:1152] = mw1a[64:128]   # z half, forward
    wpack[64:128, 1024:1152] = rw1a[64:128]  # z half, reverse
    wpack[:, 1152:1216] = f("mw1b")
    wpack[:, 1216:1280] = f("rw1b")
    w2 = f("fc2_w")
    wpack[:, 1280:1296] = w2[0:128]
    wpack[0:64, 1296:1312] = w2[128:192]    # s0f
    wpack[64:128, 1296:1312] = w2[256:320]  # s0r
    wpack[0:64, 1312:1328] = w2[192:256]    # s1f
    wpack[64:128, 1312:1328] = w2[320:384]  # s1r
    wpack[0:16, 1328:1344] = np.eye(NCLS, dtype=_F32)
    wpack[:, 1344] = f("fc1_b")
    wpack[0:NCLS, 1345] = f("fc2_b")

    in_maps = []
    for c in range(NCORES):
        b = c * BLK
        idx = (b - HALO + np.arange(W)) % N
        xt = np.ascontiguousarray(sf[idx].T).astype(bf16np)
        me = np.zeros(128 * NT, dtype=_F32)
        me[:BLK] = maskf[(b + np.arange(BLK)) % N]
        wc = wpack.copy()
        wc[:, 1346:1346 + NT] = me.reshape(NT, 128).T
        in_maps.append({"wpack": wc.astype(bf16np), "xt": xt})
    return in_maps


_RUNNER = None


def _make_runner():
    """Build the 8-core PJRT executor once; reuse across kernel() calls."""
    import jax
    from jax.sharding import Mesh, NamedSharding, PartitionSpec
    from jax.experimental.shard_map import shard_map
    from concourse import mybir
    from concourse.bass2jax import (_bass_exec_p, install_neuronx_cc_hook,
                                    partition_id_tensor)

    nc = _get_nc()
    install_neuronx_cc_hook()
    in_names, out_names, out_avals, zero_shapes = [], [], [], []
    pname = nc.partition_id_tensor.name if nc.partition_id_tensor else None
    for alloc in nc.m.functions[0].allocations:
        if not isinstance(alloc, mybir.MemoryLocationSet):
            continue
        name = alloc.memorylocations[0].name
        if alloc.kind == "ExternalInput":
            if name != pname:
                in_names.append(name)
        elif alloc.kind == "ExternalOutput":
            out_names.append(name)
            shape = tuple(alloc.tensor_shape)
            dtype = mybir.dt.np(alloc.dtype)
            out_avals.append(jax.core.ShapedArray(shape, dtype))
            zero_shapes.append((shape, dtype))
    n_params = len(in_names)
    all_in = list(in_names) + list(out_names)
    if pname is not None:
        all_in.append(pname)
    donate = tuple(range(n_params, n_params + len(out_names)))

    def _body(*args):
        operands = list(args)
        if pname is not None:
            operands.append(partition_id_tensor())
        return tuple(_bass_exec_p.bind(
            *operands,
            out_avals=tuple(out_avals),
            in_names=tuple(all_in),
            out_names=tuple(out_names),
            lowering_input_output_aliases=(),
            sim_require_finite=True,
            sim_require_nnan=True,
            nc=nc,
        ))

    devices = jax.devices()[:NCORES]
    mesh = Mesh(np.asarray(devices), ("core",))
    shd = NamedSharding(mesh, PartitionSpec("core"))
    n_outs = len(out_names)
    sharded = jax.jit(
        shard_map(_body, mesh=mesh,
                  in_specs=(PartitionSpec("core"),) * (n_params + n_outs),
                  out_specs=(PartitionSpec("core"),) * n_outs,
                  check_rep=False),
        donate_argnums=donate, keep_unused=True,
    )

    def run(in_maps):
        concat_in = [
            np.concatenate([np.asarray(in_maps[c][nm]) for c in range(NCORES)],
                           axis=0)
            for nm in in_names
        ]
        dev_in = [jax.device_put(x, shd) for x in concat_in]
        zeros = [
            jax.device_put(np.zeros((NCORES * s[0], *s[1:]), dt), shd)
            for s, dt in zero_shapes
        ]
        outs = sharded(*dev_in, *zeros)
        res = np.asarray(outs[out_names.index("out")])
        return np.ascontiguousarray(res.reshape(NCORES * BLK, NCLS))

    return run


def kernel(**inputs):
    if not _structure_matches(inputs):
        return _reference_numpy(inputs)
    global _RUNNER
    if _RUNNER is None:
        _RUNNER = _make_runner()
    return _RUNNER(_make_in_maps(inputs))

